# revision 1
# baseline (speedup 1.0000x reference)
"""Trainium2 Bass kernel for the controlled-unitary problem.

reference semantics (control=0, num_qubits=13, dim=8192):
    mask bit = 1 << 12, so columns/rows with that bit set are idx 4096..8191.
    out[:, c0] = state[:, c0]                       (control bit off: untouched)
    out[:, c1] = state[:, c1] @ target[c1, c1]      (controlled unitary)

Device work: complex [256,4096] @ [4096,4096] GEMM = 4 real GEMMs.
Sharding: output columns of the GEMM split 8 ways (each core gets a
[4096, 512] slab of the target block; every weight byte moves once).

Per-core kernel (v2):
  - A planes a_r, a_i and a_n = -a_i (negation host-side) let the real
    part accumulate directly in PSUM: bank_r += a_r.b_r + a_n.b_i,
    bank_i += a_r.b_i + a_i.b_r  ->  4 PSUM banks (2 M-tiles x re/im),
    combine is just a PSUM->SBUF copy.
  - DMA on both HWDGE rings: A planes + outputs on nc.sync (SP ring),
    B planes on nc.scalar (ACT ring).
  - K streamed in ramped chunks (small first chunk so the PE starts
    early, big later chunks for DMA efficiency).
"""

import os

import numpy as np

BATCH = 256
DIM = 8192
HALF = 4096
N_CORES = 8
NSH = HALF // N_CORES  # 512 output columns per core
KT = HALF // 128  # 32 k-tiles
MT = BATCH // 128  # 2 m-tiles
CHUNKS = [1, 1, 2, 4, 8, 8, 8]  # k-tiles per DMA chunk (sums to KT)
CHMAX = max(CHUNKS)

# matmul dtype: "float32r" = full-rate fp32 path, "float16" = half traffic
DT_NAME = os.environ.get("KERNEL_DT", "float16")

_CACHE = {}


def _np_dtype(dt_name):
    return np.float16 if dt_name == "float16" else np.float32


def _build(dt_name):
    import concourse.mybir as mybir
    import concourse.tile as tile
    from concourse import bacc

    DT = getattr(mybir.dt, dt_name)
    F32 = mybir.dt.float32

    nc = bacc.Bacc("TRN2", target_bir_lowering=False, debug=False,
                   num_devices=N_CORES)

    a_r = nc.dram_tensor("a_r", [128, KT, BATCH], DT, kind="ExternalInput")
    a_i = nc.dram_tensor("a_i", [128, KT, BATCH], DT, kind="ExternalInput")
    b_r = nc.dram_tensor("b_r", [128, KT, NSH], DT, kind="ExternalInput")
    b_i = nc.dram_tensor("b_i", [128, KT, NSH], DT, kind="ExternalInput")
    c_r = nc.dram_tensor("c_r", [BATCH, NSH], F32, kind="ExternalOutput")
    c_i = nc.dram_tensor("c_i", [BATCH, NSH], F32, kind="ExternalOutput")

    with tile.TileContext(nc) as tc:
        with (
            tc.tile_pool(name="ap", bufs=4) as ap_pool,
            tc.tile_pool(name="bp", bufs=4) as bp_pool,
            tc.tile_pool(name="op", bufs=2) as o_pool,
            tc.tile_pool(name="ps", bufs=1, space="PSUM") as ps_pool,
        ):
            # Gauss 3-multiplication complex GEMM:
            #   k1 = (a_r+a_i).b_r   k2 = a_r.(b_i-b_r)   k3n = (-a_i).(b_r+b_i)
            #   C_r = k1 + k3n       C_i = k1 + k2
            ps = {}
            for m in range(MT):
                for comp in ("k1", "k2", "k3"):
                    ps[(m, comp)] = ps_pool.tile(
                        [128, NSH], F32, name=f"ps_{m}_{comp}"
                    )

            k0 = 0
            for ch in CHUNKS:
                nb = 3 if ch == CHMAX else 2
                ar_t = ap_pool.tile([128, ch, BATCH], DT, name=f"ar{ch}", bufs=nb)
                ai_t = ap_pool.tile([128, ch, BATCH], DT, name=f"ai{ch}", bufs=nb)
                as_t = ap_pool.tile([128, ch, BATCH], DT, name=f"as{ch}", bufs=nb)
                br_t = bp_pool.tile([128, ch, NSH], DT, name=f"br{ch}", bufs=nb)
                bi_t = bp_pool.tile([128, ch, NSH], DT, name=f"bi{ch}", bufs=nb)
                bs_t = bp_pool.tile([128, ch, NSH], DT, name=f"bs{ch}", bufs=nb)
                ksl = slice(k0, k0 + ch)
                # two HWDGE rings, balanced: SP ring gets a_r + b_r,
                # ACT ring gets a_i + b_i (6.3MB each)
                nc.sync.dma_start(ar_t[:], a_r[:, ksl, :])
                nc.scalar.dma_start(ai_t[:], a_i[:, ksl, :])
                nc.sync.dma_start(br_t[:], b_r[:, ksl, :])
                nc.scalar.dma_start(bi_t[:], b_i[:, ksl, :])
                # DVE operand prep (fp16 SBUF 2x/4x modes, overlapped with PE):
                #   as = a_r + a_i;  ai <- -a_i (in place, becomes a_n)
                #   bs = b_r + b_i;  bi <- b_i - b_r (in place, becomes b_d)
                nc.vector.tensor_tensor(as_t[:], ar_t[:], ai_t[:],
                                        mybir.AluOpType.add)
                nc.vector.tensor_scalar_mul(ai_t[:], ai_t[:], -1.0)
                nc.vector.tensor_tensor(bs_t[:], br_t[:], bi_t[:],
                                        mybir.AluOpType.add)
                nc.vector.tensor_tensor(bi_t[:], bi_t[:], br_t[:],
                                        mybir.AluOpType.subtract)
                last_chunk = k0 + ch == KT
                # product-major order inside the chunk: k1 matmuls only
                # depend on the `as` prep, so the PE starts them while DVE
                # still computes bs/bd for k2/k3
                operands = {
                    "k1": (as_t, br_t),
                    "k2": (ar_t, bi_t),
                    "k3": (ai_t, bs_t),
                }
                for comp in ("k1", "k2", "k3"):
                    lhs_t, rhs_t = operands[comp]
                    for m in range(MT):
                        for kk in range(ch):
                            k = k0 + kk
                            msl = slice(m * 128, (m + 1) * 128)
                            nc.tensor.matmul(
                                ps[(m, comp)][:], lhs_t[:, kk, msl],
                                rhs_t[:, kk, :], start=(k == 0),
                                stop=(last_chunk and kk == ch - 1),
                            )
                k0 += ch

            for m in range(MT):
                msl = slice(m * 128, (m + 1) * 128)
                t2 = o_pool.tile([128, NSH], F32, name="t2")
                t3 = o_pool.tile([128, NSH], F32, name="t3")
                out_r = o_pool.tile([128, NSH], F32, name="out_r")
                out_i = o_pool.tile([128, NSH], F32, name="out_i")
                nc.vector.tensor_copy(t3[:], ps[(m, "k3")][:])
                nc.vector.tensor_copy(t2[:], ps[(m, "k2")][:])
                nc.vector.tensor_tensor(out_r[:], ps[(m, "k1")][:], t3[:],
                                        mybir.AluOpType.add)
                nc.vector.tensor_tensor(out_i[:], ps[(m, "k1")][:], t2[:],
                                        mybir.AluOpType.add)
                nc.sync.dma_start(c_r[msl, :], out_r[:])
                nc.scalar.dma_start(c_i[msl, :], out_i[:])

    nc.compile()
    return nc


def _get_nc(dt_name):
    if dt_name not in _CACHE:
        _CACHE[dt_name] = _build(dt_name)
    return _CACHE[dt_name]


def _pack_kxm(mat_t, np_dt):
    # mat_t: [4096, F] (k-major) -> [128, KT, F] with k = kt*128 + p
    f = mat_t.shape[1]
    return np.ascontiguousarray(
        mat_t.reshape(KT, 128, f).transpose(1, 0, 2).astype(np_dt)
    )


def run_device(A, B, dt_name=DT_NAME, trace=False):
    """A: [256, 4096] complex64, B: [4096, 4096] complex64.
    Returns C = A @ B as [256, 4096] complex64 plus the raw results."""
    from concourse import bass_utils

    nc = _get_nc(dt_name)
    np_dt = _np_dtype(dt_name)

    at = A.T  # [4096, 256]
    a_r = _pack_kxm(np.ascontiguousarray(at.real), np_dt)
    a_i = _pack_kxm(np.ascontiguousarray(at.imag), np_dt)
    br_full = B.real
    bi_full = B.imag

    in_maps = []
    for c in range(N_CORES):
        csl = slice(c * NSH, (c + 1) * NSH)
        in_maps.append({
            "a_r": a_r,
            "a_i": a_i,
            "b_r": _pack_kxm(np.ascontiguousarray(br_full[:, csl]), np_dt),
            "b_i": _pack_kxm(np.ascontiguousarray(bi_full[:, csl]), np_dt),
        })

    res = bass_utils.run_bass_kernel_spmd(
        nc, in_maps, core_ids=list(range(N_CORES)), trace=trace
    )

    out = np.empty((BATCH, HALF), dtype=np.complex64)
    for c in range(N_CORES):
        csl = slice(c * NSH, (c + 1) * NSH)
        out.real[:, csl] = res.results[c]["c_r"]
        out.imag[:, csl] = res.results[c]["c_i"]
    return out, res


def kernel(state, target_matrix, control, num_qubits):
    state = np.asarray(state)
    target_matrix = np.asarray(target_matrix)
    control = int(control)
    num_qubits = int(num_qubits)
    dim = 1 << num_qubits

    assert state.shape == (BATCH, DIM) and dim == DIM, (
        "kernel hardcoded for [256, 8192]"
    )

    mask = 1 << (num_qubits - control - 1)
    idx = np.arange(dim)
    c1 = idx[(idx & mask) != 0]  # columns with control bit set

    if control == 0:
        A = state[:, HALF:]
        B = target_matrix[HALF:, HALF:]
    else:
        A = state[:, c1]
        B = target_matrix[np.ix_(c1, c1)]
    A = np.ascontiguousarray(A, dtype=np.complex64)
    B = np.ascontiguousarray(B, dtype=np.complex64)

    C, _ = run_device(A, B)

    out = state.astype(np.complex64, copy=True)
    out[:, c1] = C
    return out



# revision 2
# speedup vs baseline: 1.2042x; 1.2042x over previous
"""Trainium2 Bass kernel for the controlled-unitary problem.

reference semantics (control=0, num_qubits=13, dim=8192):
    mask bit = 1 << 12, so columns/rows with that bit set are idx 4096..8191.
    out[:, c0] = state[:, c0]                       (control bit off: untouched)
    out[:, c1] = state[:, c1] @ target[c1, c1]      (controlled unitary)

Device work: complex [256,4096] @ [4096,4096] GEMM.
Sharding: output columns of the GEMM split 8 ways (each core gets a
[4096, 512] slab of the target block; every weight byte moves once).

Per-core kernel (v3):
  - Karatsuba split: t1 = ar.br, t2 = ai.bi, t3 = (ar+ai).(br+bi);
    C_r = t1 - t2, C_i = t3 - t1 - t2. Only ONE derived plane per
    operand (as, bs) -> half the DVE prep of the Gauss variant.
  - All inputs fully resident in SBUF (~150KB/partition): every DMA is
    issued upfront with no buffer recycling, so the two HWDGE rings
    stream back-to-back at full rate.
  - Warmup matmuls on a zeroed scratch tile during the DMA head keep
    the PE p-state ramping so real matmuls run at full clock.
  - Last chunk is m-major so m0's epilogue overlaps m1's matmuls.
"""

import os

import numpy as np

BATCH = 256
DIM = 8192
HALF = 4096
N_CORES = 8
NSH = HALF // N_CORES  # 512 output columns per core
KT = HALF // 128  # 32 k-tiles
MT = BATCH // 128  # 2 m-tiles

DT_NAME = os.environ.get("KERNEL_DT", "float16")
CHUNKS = [int(x) for x in os.environ.get(
    "KERNEL_CHUNKS", "1,1,2,4,8,8,8").split(",")]
assert sum(CHUNKS) == KT
NWARM = int(os.environ.get("KERNEL_NWARM", "12"))

_CACHE = {}


def _np_dtype(dt_name):
    return np.float16 if dt_name == "float16" else np.float32


def _build(dt_name):
    import concourse.mybir as mybir
    import concourse.tile as tile
    from concourse import bacc

    DT = getattr(mybir.dt, dt_name)
    F32 = mybir.dt.float32

    nc = bacc.Bacc("TRN2", target_bir_lowering=False, debug=False,
                   num_devices=N_CORES)

    a_r = nc.dram_tensor("a_r", [128, KT, BATCH], DT, kind="ExternalInput")
    a_i = nc.dram_tensor("a_i", [128, KT, BATCH], DT, kind="ExternalInput")
    b_r = nc.dram_tensor("b_r", [128, KT, NSH], DT, kind="ExternalInput")
    b_i = nc.dram_tensor("b_i", [128, KT, NSH], DT, kind="ExternalInput")
    c_r = nc.dram_tensor("c_r", [BATCH, NSH], F32, kind="ExternalOutput")
    c_i = nc.dram_tensor("c_i", [BATCH, NSH], F32, kind="ExternalOutput")

    with tile.TileContext(nc) as tc:
        with (
            tc.tile_pool(name="sb", bufs=1) as sb,
            tc.tile_pool(name="ps", bufs=1, space="PSUM") as ps_pool,
        ):
            A_r = sb.tile([128, KT, BATCH], DT, name="A_r")
            A_i = sb.tile([128, KT, BATCH], DT, name="A_i")
            A_s = sb.tile([128, KT, BATCH], DT, name="A_s")
            B_r = sb.tile([128, KT, NSH], DT, name="B_r")
            B_i = sb.tile([128, KT, NSH], DT, name="B_i")
            B_s = sb.tile([128, KT, NSH], DT, name="B_s")
            warm_w = sb.tile([128, 128], DT, name="warm_w")
            warm_x = sb.tile([128, NSH], DT, name="warm_x")

            ps = {}
            for m in range(MT):
                for t in ("t1", "t2", "t3"):
                    ps[(m, t)] = ps_pool.tile([128, NSH], F32,
                                              name=f"ps_{m}_{t}")
            ps_warm = ps_pool.tile([128, NSH], F32, name="ps_warm")

            # PE warmup: ramp the p-state while input DMA streams in.
            nc.vector.memset(warm_w[:], 0.0)
            nc.vector.memset(warm_x[:], 0.0)
            for _ in range(NWARM):
                nc.tensor.matmul(ps_warm[:], warm_w[:], warm_x[:],
                                 start=True, stop=True)

            # operand streams: ring assignment is balanced so that each
            # product's pair lands first on its two rings:
            #   SP ring:  A_r then B_i     ACT ring: B_r then A_i
            k0 = 0
            n_chunks = len(CHUNKS)
            for ci, ch in enumerate(CHUNKS):
                ksl = slice(k0, k0 + ch)
                nc.sync.dma_start(A_r[:, ksl], a_r[:, ksl])
                nc.scalar.dma_start(B_r[:, ksl], b_r[:, ksl])
                nc.sync.dma_start(B_i[:, ksl], b_i[:, ksl])
                nc.scalar.dma_start(A_i[:, ksl], a_i[:, ksl])
                nc.vector.tensor_tensor(A_s[:, ksl], A_r[:, ksl],
                                        A_i[:, ksl], mybir.AluOpType.add)
                nc.vector.tensor_tensor(B_s[:, ksl], B_r[:, ksl],
                                        B_i[:, ksl], mybir.AluOpType.add)

                operands = {
                    "t1": (A_r, B_r),
                    "t2": (A_i, B_i),
                    "t3": (A_s, B_s),
                }
                last_chunk = ci == n_chunks - 1
                if not last_chunk:
                    # product-major: t1 only needs the first transfer of
                    # each ring, so the PE starts before A_i/B_i land
                    for t in ("t1", "t2", "t3"):
                        lhs, rhs = operands[t]
                        for kk in range(ch):
                            k = k0 + kk
                            for m in range(MT):
                                msl = slice(m * 128, (m + 1) * 128)
                                nc.tensor.matmul(
                                    ps[(m, t)][:], lhs[:, k, msl],
                                    rhs[:, k, :], start=(k == 0),
                                    stop=False,
                                )
                else:
                    # m-major: finish all of m0 first so its epilogue
                    # overlaps m1's matmuls
                    for m in range(MT):
                        msl = slice(m * 128, (m + 1) * 128)
                        for t in ("t1", "t2", "t3"):
                            lhs, rhs = operands[t]
                            for kk in range(ch):
                                k = k0 + kk
                                nc.tensor.matmul(
                                    ps[(m, t)][:], lhs[:, k, msl],
                                    rhs[:, k, :], start=(k == 0),
                                    stop=(kk == ch - 1),
                                )
                        # epilogue for this m-tile:
                        #   C_r = t1 - t2, C_i = (t3 - t1) - t2
                        t1s = sb.tile([128, NSH], F32, name=f"t1s{m}")
                        out_r = sb.tile([128, NSH], F32, name=f"out_r{m}")
                        u = sb.tile([128, NSH], F32, name=f"u{m}")
                        out_i = sb.tile([128, NSH], F32, name=f"out_i{m}")
                        nc.scalar.copy(t1s[:], ps[(m, "t1")][:])
                        nc.vector.tensor_tensor(
                            out_r[:], t1s[:], ps[(m, "t2")][:],
                            mybir.AluOpType.subtract)
                        nc.vector.tensor_tensor(
                            u[:], ps[(m, "t3")][:], t1s[:],
                            mybir.AluOpType.subtract)
                        nc.vector.tensor_tensor(
                            out_i[:], u[:], ps[(m, "t2")][:],
                            mybir.AluOpType.subtract)
                        nc.sync.dma_start(c_r[msl, :], out_r[:])
                        nc.scalar.dma_start(c_i[msl, :], out_i[:])
                k0 += ch

    nc.compile()
    return nc


def _get_nc(dt_name):
    if dt_name not in _CACHE:
        _CACHE[dt_name] = _build(dt_name)
    return _CACHE[dt_name]


def _pack_kxm(mat_t, np_dt):
    # mat_t: [4096, F] (k-major) -> [128, KT, F] with k = kt*128 + p
    f = mat_t.shape[1]
    return np.ascontiguousarray(
        mat_t.reshape(KT, 128, f).transpose(1, 0, 2).astype(np_dt)
    )


def run_device(A, B, dt_name=DT_NAME, trace=False):
    """A: [256, 4096] complex64, B: [4096, 4096] complex64.
    Returns C = A @ B as [256, 4096] complex64 plus the raw results."""
    from concourse import bass_utils

    nc = _get_nc(dt_name)
    np_dt = _np_dtype(dt_name)

    at = A.T  # [4096, 256]
    a_r = _pack_kxm(np.ascontiguousarray(at.real), np_dt)
    a_i = _pack_kxm(np.ascontiguousarray(at.imag), np_dt)
    br_full = B.real
    bi_full = B.imag

    in_maps = []
    for c in range(N_CORES):
        csl = slice(c * NSH, (c + 1) * NSH)
        in_maps.append({
            "a_r": a_r,
            "a_i": a_i,
            "b_r": _pack_kxm(np.ascontiguousarray(br_full[:, csl]), np_dt),
            "b_i": _pack_kxm(np.ascontiguousarray(bi_full[:, csl]), np_dt),
        })

    res = bass_utils.run_bass_kernel_spmd(
        nc, in_maps, core_ids=list(range(N_CORES)), trace=trace
    )

    out = np.empty((BATCH, HALF), dtype=np.complex64)
    for c in range(N_CORES):
        csl = slice(c * NSH, (c + 1) * NSH)
        out.real[:, csl] = res.results[c]["c_r"]
        out.imag[:, csl] = res.results[c]["c_i"]
    return out, res


def kernel(state, target_matrix, control, num_qubits):
    state = np.asarray(state)
    target_matrix = np.asarray(target_matrix)
    control = int(control)
    num_qubits = int(num_qubits)
    dim = 1 << num_qubits

    assert state.shape == (BATCH, DIM) and dim == DIM, (
        "kernel hardcoded for [256, 8192]"
    )

    mask = 1 << (num_qubits - control - 1)
    idx = np.arange(dim)
    c1 = idx[(idx & mask) != 0]  # columns with control bit set

    if control == 0:
        A = state[:, HALF:]
        B = target_matrix[HALF:, HALF:]
    else:
        A = state[:, c1]
        B = target_matrix[np.ix_(c1, c1)]
    A = np.ascontiguousarray(A, dtype=np.complex64)
    B = np.ascontiguousarray(B, dtype=np.complex64)

    C, _ = run_device(A, B)

    out = state.astype(np.complex64, copy=True)
    out[:, c1] = C
    return out
